# revision 19
# baseline (speedup 1.0000x reference)
"""GTrXL layer on 8 TRN2 NeuronCores — fp8-DoubleRow rewrite.

Sharding: pure data-parallel over batch (BS=8 -> 1 batch element/core),
no collectives.

Key points vs the bf16 baseline:
- All big GEMMs (kv/q/pos, proj, 12 GRU gates, MLP) run in fp8e4 with
  MatmulPerfMode.DoubleRow: 2 K-chunks of 128 contract per instruction at
  0.5 cycles/row -> 4x fewer PE cycles than bf16 chains.
- Uniform fp8 scales: activations x8, weights x256 (descale 2^-11 in the
  epilogues, folded into activation scale/bias immediates).
- Attention scores stay bf16 (head_dim=64 contraction can't DoubleRow).
  Rel-shift runs through a DRAM scratch in fp8: pos scores [q, relk] are
  staged to scr rows (tail 512 cols prefilled with -192), the shifted
  window read back [q, keys]; a plain fp8 identity matmul loads the
  shifted scores into PSUM (start=True) and the trimmed content matmuls
  accumulate on top, so masking comes free via the pad.
- exp on Act engine writes attn weights straight to fp8 (scale 1/4 via
  bias=-ln4); fp8 transposes put them in [keys, queries]; denominator via
  a 0.5-valued-ones DoubleRow matmul; AV in fp8 DoubleRow.
- LN stats via bf16 ones-matmuls; x-residual path of the GRUs in f32.
- Input x^T / pos^T / inputs^T pre-transposed on host; output returned
  transposed [D, CUR] and untransposed on host.
"""

import sys

if '/opt/trn_rl_repo' not in sys.path:
    sys.path.insert(0, '/opt/trn_rl_repo')

import numpy as np
import ml_dtypes

import concourse.bass as bass
import concourse.tile as tile
from concourse import bacc, mybir
from concourse.bass_utils import run_bass_kernel_spmd
from concourse.masks import make_identity

BF16 = mybir.dt.bfloat16
F32 = mybir.dt.float32
F8 = mybir.dt.float8e4
NF8 = ml_dtypes.float8_e4m3
NBF = ml_dtypes.bfloat16
DR = mybir.MatmulPerfMode.DoubleRow

HEAD_NUM, HEAD_DIM = 16, 64
D, HID = 1024, 4096
CUR, PREV, BS = 512, 512, 8
FULL = CUR + PREV
EPS = 1e-5
SCALE = 1.0 / (HEAD_DIM ** 0.5)
P = 128
DC = D // P          # 8 feature chunks
HC = HID // P        # 32 hidden chunks
TCF = FULL // P      # 8 full-token chunks
TCC = CUR // P       # 4 query-token chunks

SA = 8.0             # fp8 activation scale
SW = 256.0           # fp8 weight scale
DS = 1.0 / (SA * SW)     # gemm descale 2^-11
SV = 8.0             # fp8 V scale
S_AV = 2.0 * SV      # av fp8 scale (from 0.5-ones denominator trick)
PAD = -192.0         # fp8 pad value for masked rel-shift region
LN4 = float(np.log(4.0))

AluOp = mybir.AluOpType
Act = mybir.ActivationFunctionType


def _dram_in(dram, name, shape, dtype):
    return dram.tile(list(shape), dtype, kind="ExternalInput", name=name,
                     uniquify=False)


def _dr_chain(nc, psum, w, act, n0, n1, nk, t0, t1):
    """psum[:, :] (+)= sum_k w[:, k, n0:n1].T @ act[:, k, t0:t1] via DoubleRow
    over nk k-chunks (nk even)."""
    for i in range(nk // 2):
        nc.tensor.matmul(psum, lhsT=w[:, 2 * i:2 * i + 2, n0:n1],
                         rhs=act[:, 2 * i:2 * i + 2, t0:t1],
                         start=(i == 0), stop=(i == nk // 2 - 1), perf_mode=DR)


def _build():
    nc = bacc.Bacc("TRN2", target_bir_lowering=False)
    with tile.TileContext(nc) as tc:
        _emit(nc, tc)
    nc.compile()
    return nc


def _emit(nc, tc):
    from contextlib import ExitStack

    with ExitStack() as root:
        dram = root.enter_context(tc.tile_pool(name="io", bufs=1, space="DRAM"))

        # ---------------- DRAM I/O ----------------
        xT_d = _dram_in(dram, "xT_b", (D, FULL), BF16)
        inpTb_d = _dram_in(dram, "inpT_b", (D, CUR), BF16)
        inpT8_d = _dram_in(dram, "inpT8", (D, CUR), F8)
        posT8_d = _dram_in(dram, "posT8", (D, FULL), F8)

        CLAYOUT = [("ln1gs", DC), ("ln1bs", DC), ("ln2gs", DC),
                   ("ln2bs", DC), ("bkvK", DC), ("bqu", DC), ("bqv", DC),
                   ("bpos", DC), ("bprojs", DC), ("b1s", HC), ("b2s", DC),
                   ("nbg1", DC), ("nbg2", DC), ("dvu", DC)]
        CW = sum(w for _, w in CLAYOUT)
        cmerged_d = _dram_in(dram, "cmerged", (P, CW), F32)

        wkv_d = _dram_in(dram, "Wkv8", (D, 2 * D), F8)
        wq_d = _dram_in(dram, "Wq8", (D, D), F8)
        wpos_d = _dram_in(dram, "Wpos8", (D, D), F8)
        wproj_d = _dram_in(dram, "Wproj8", (D, D), F8)
        gw_d = {}
        for g in (1, 2):
            for m in ("Wr", "Ur", "Wz", "Uz", "Wg", "Ug"):
                gw_d[(g, m)] = _dram_in(dram, f"g{g}_{m}8", (D, D), F8)
        w1_d = _dram_in(dram, "mlp_W18", (D, HID), F8)
        w2_d = _dram_in(dram, "mlp_W28", (HID, D), F8)

        outT_d = dram.tile([D, CUR], F32, kind="ExternalOutput", name="outT",
                           uniquify=False)

        n_scr = 4
        # per slot: 512 q-rows x 1536 (1024 scores + 512 pad)
        scr = [dram.tile([P, TCC, 1536], F8, name=f"scr{s}") for s in range(n_scr)]

        # ---------------- constants ----------------
        const = root.enter_context(tc.tile_pool(name="const", bufs=1))
        identb = const.tile([P, P], BF16)
        make_identity(nc, identb)
        ident8 = const.tile([P, P], F8)
        nc.vector.tensor_copy(ident8, identb)
        ones_b = const.tile([P, 1], BF16)
        nc.vector.memset(ones_b, 1.0)
        oneshalf8 = const.tile([P, 2, 64], F8)
        nc.vector.memset(oneshalf8, 0.5)
        eps_t = const.tile([1, 1], F32)
        nc.vector.memset(eps_t, EPS)
        zerob = const.tile([P, 1], F32)
        nc.vector.memset(zerob, 0.0)
        ln4n = const.tile([P, 1], F32)
        nc.vector.memset(ln4n, -LN4)

        cmg = const.tile([P, CW], F32, name="cmg")
        nc.sync.dma_start(out=cmg, in_=cmerged_d[:])
        csb = {"zerob": zerob}
        off = 0
        for name, w in CLAYOUT:
            csb[name] = cmg[:, off:off + w]
            off += w

        # ---------------- psum pools (7 + 1 = 8 banks) ----------------
        psum = root.enter_context(tc.tile_pool(name="psA", bufs=7, space="PSUM"))
        psum_m = root.enter_context(tc.tile_pool(name="pm", bufs=1, space="PSUM"))

        def PS():
            return psum.tile([P, 512], F32, name="ps", tag="ps")

        def mk(name, shape, dtype, side):
            t, fr = tc.tile(list(shape), dtype, name=name, side=side)
            return t, fr

        # ---- long-lived left-side tiles, allocated in lifetime order
        # (die-last at the bottom of the stack); DMAs issued separately ----
        av8, fr_av8 = mk("av8", (P, DC, CUR), F8, "left")   # freed at end
        wkv, fr_wkv = mk("wkv", (P, DC, 2 * D), F8, "left")
        x1T8, fr_x1T8 = mk("x1T8", (P, DC, FULL), F8, "left")
        posT8, fr_posT8 = mk("posT8s", (P, DC, FULL), F8, "left")
        wpos, fr_wpos = mk("wpos", (P, DC, D), F8, "left")
        wq, fr_wq = mk("wq", (P, DC, D), F8, "left")

        # ================= Phase 1: LN1 (transposed domain) =================
        with ExitStack() as ph:
            xw = ph.enter_context(tc.tile_pool(name="xw", bufs=1, side="left"))
            lw = ph.enter_context(tc.tile_pool(name="lw1", bufs=2, side="left"))
            xT = xw.tile([P, DC, FULL], BF16)
            xTr = xT_d[:].rearrange("(dc p) t -> p dc t", p=P)
            for dc in range(DC):
                nc.sync.dma_start(out=xT[:, dc, :], in_=xTr[:, dc, :])
            nc.sync.dma_start(out=wkv, in_=wkv_d[:].rearrange("(kc p) n -> p kc n", p=P))
            nc.sync.dma_start(out=wq, in_=wq_d[:].rearrange("(kc p) n -> p kc n", p=P))
            nc.sync.dma_start(out=wpos, in_=wpos_d[:].rearrange("(kc p) n -> p kc n", p=P))
            nc.sync.dma_start(out=posT8, in_=posT8_d[:].rearrange("(kc p) f -> p kc f", p=P))
            sq = xw.tile([P, DC, FULL], BF16)
            for dc in range(DC):
                nc.gpsimd.tensor_tensor(out=sq[:, dc, :], in0=xT[:, dc, :],
                                        in1=xT[:, dc, :], op=AluOp.mult)
            mean = lw.tile([1, FULL], F32, name="mean")
            m2m = lw.tile([1, FULL], F32, name="m2m")
            for th in range(2):
                s1p = psum_m.tile([1, 512], F32, name="s1p", tag="sm")
                for dc in range(DC):
                    nc.tensor.matmul(s1p, lhsT=ones_b,
                                     rhs=xT[:, dc, th * 512:(th + 1) * 512],
                                     start=(dc == 0), stop=(dc == DC - 1))
                nc.vector.tensor_scalar_mul(mean[:, th * 512:(th + 1) * 512], s1p,
                                            1.0 / D)
                s2p = psum_m.tile([1, 512], F32, name="s2p", tag="sm")
                for dc in range(DC):
                    nc.tensor.matmul(s2p, lhsT=ones_b,
                                     rhs=sq[:, dc, th * 512:(th + 1) * 512],
                                     start=(dc == 0), stop=(dc == DC - 1))
                nc.vector.tensor_scalar_mul(m2m[:, th * 512:(th + 1) * 512], s2p,
                                            1.0 / D)
            var = lw.tile([1, FULL], F32, name="var")
            nc.vector.scalar_tensor_tensor(out=var, in0=mean, scalar=1.0,
                                           in1=mean, op0=AluOp.mult, op1=AluOp.mult)
            nc.vector.tensor_sub(var, m2m, var)
            sd = lw.tile([1, FULL], F32, name="sd")
            nc.scalar.activation(out=sd, in_=var, func=Act.Sqrt, bias=eps_t)
            rstd = lw.tile([1, FULL], F32, name="rstd")
            nc.vector.reciprocal(out=rstd, in_=sd)
            mean_b = lw.tile([1, FULL], BF16, name="mean_b")
            nc.vector.tensor_copy(mean_b, mean)
            rstd_b = lw.tile([1, FULL], BF16, name="rstd_b")
            nc.vector.tensor_copy(rstd_b, rstd)
            meanB = lw.tile([P, FULL], BF16, name="meanB")
            nc.gpsimd.partition_broadcast(meanB, mean_b)
            rstdB = lw.tile([P, FULL], BF16, name="rstdB")
            nc.gpsimd.partition_broadcast(rstdB, rstd_b)
            tw = ph.enter_context(tc.tile_pool(name="tw", bufs=3, side="left"))
            for dc in range(DC):
                t1 = tw.tile([P, FULL], BF16, name="t1")
                nc.vector.tensor_sub(t1, xT[:, dc, :], meanB)
                t2 = tw.tile([P, FULL], BF16, name="t2")
                nc.vector.tensor_tensor(out=t2, in0=t1, in1=rstdB, op=AluOp.mult)
                nc.scalar.activation(out=x1T8[:, dc, :], in_=t2, func=Act.Identity,
                                     bias=csb["ln1bs"][:, dc:dc + 1],
                                     scale=csb["ln1gs"][:, dc:dc + 1])

        # ================= Phase 2: kT, v8, quT/qvT, rT =================
        kT, fr_kT = mk("kT", (P, DC, FULL), BF16, "right")
        v8, fr_v8 = mk("v8", (P, TCF, D), F8, "right")
        quT, fr_quT = mk("quT", (P, DC, CUR), BF16, "right")
        qvT, fr_qvT = mk("qvT", (P, DC, CUR), BF16, "right")
        rT, fr_rT = mk("rT", (P, DC, FULL), BF16, "right")

        with ExitStack() as ph:
            for n in range(DC):
                ps = PS()
                _dr_chain(nc, ps, wq, x1T8, n * P, (n + 1) * P, DC, CUR, FULL)
                nc.vector.tensor_scalar(out=quT[:, n, :], in0=ps, scalar1=DS,
                                        scalar2=csb["bqu"][:, n:n + 1],
                                        op0=AluOp.mult, op1=AluOp.add)
                nc.gpsimd.tensor_scalar(out=qvT[:, n, :], in0=quT[:, n, :],
                                        scalar1=csb["dvu"][:, n:n + 1],
                                        scalar2=0.0, op0=AluOp.add, op1=AluOp.add)
            fr_wq()
            for n in range(DC):
                for fh in range(2):
                    ps = PS()
                    _dr_chain(nc, ps, wpos, posT8, n * P, (n + 1) * P, DC,
                              fh * 512, (fh + 1) * 512)
                    nc.scalar.activation(out=rT[:, n, fh * 512:(fh + 1) * 512],
                                         in_=ps, func=Act.Identity,
                                         bias=csb["bpos"][:, n:n + 1], scale=DS)
            fr_wpos(); fr_posT8()
            aw = ph.enter_context(tc.tile_pool(name="aw", bufs=4, side="right"))
            ew = ph.enter_context(tc.tile_pool(name="ew", bufs=3, side="right"))
            rw = ph.enter_context(tc.tile_pool(name="rw", bufs=3, side="right"))
            padw8 = const.tile([P, TCC, 512], F8)
            nc.vector.memset(padw8, PAD)
            for sl in range(n_scr):
                pad_ap = bass.AP(tensor=scr[sl].tensor, offset=scr[sl].offset + 1024,
                                 ap=[[1536, P], [1536 * P, TCC], [1, 512]])
                nc.sync.dma_start(out=pad_ap, in_=padw8)
    
            def emit_pos(h):
                ch, rb = h // 2, (h % 2) * HEAD_DIM
                qvh = qvT[rb:rb + HEAD_DIM, ch, :]
                rh = rT[rb:rb + HEAD_DIM, ch, :]
                s_t = scr[h % n_scr]
                pb = aw.tile([P, TCC, FULL], F8, name="pb")
                for ic in range(TCC):
                    # rows i in chunk ic only read cols c >= 384 - ic*128
                    c0 = 384 - ic * P
                    for jh in range(2):
                        j0 = max(jh * 512, c0)
                        j1 = (jh + 1) * 512
                        pp = PS()
                        nc.tensor.matmul(pp[:, j0 - jh * 512:512],
                                         lhsT=qvh[:, ic * P:(ic + 1) * P],
                                         rhs=rh[:, j0:j1], start=True, stop=True)
                        dst = pb[:, ic, j0:j1]
                        src = pp[:, j0 - jh * 512:512]
                        if jh == 0:
                            # trimmed (smaller) slabs go to the hotter Act
                            nc.scalar.activation(out=dst, in_=src,
                                                 func=Act.Identity,
                                                 bias=csb["zerob"], scale=1.0)
                        else:
                            nc.vector.tensor_copy(dst, src)
                for ic in range(TCC):
                    c0 = 384 - ic * P
                    wr_ap = bass.AP(
                        tensor=s_t.tensor,
                        offset=s_t.offset + ic * P * 1536 + c0,
                        ap=[[1536, P], [1, FULL - c0]])
                    nc.sync.dma_start(out=wr_ap, in_=pb[:, ic, c0:])
                shp = aw.tile([P, TCC, FULL], F8, name="shp")
                rd_ap = bass.AP(tensor=s_t.tensor, offset=s_t.offset + 511,
                                ap=[[1535, P], [1535 * P, TCC], [1, FULL]])
                nc.sync.dma_start(out=shp, in_=rd_ap)
                return shp

            def emit_rest(h, shp):
                ch, rb = h // 2, (h % 2) * HEAD_DIM
                quh = quT[rb:rb + HEAD_DIM, ch, :]
                kh = kT[rb:rb + HEAD_DIM, ch, :]
                # scores directly in [keys, queries] layout: content via
                # lhsT=k-chunk, shifted pos scores via fp8 transpose-load
                # matmuls (lhsT=shp tile, rhs=identity) accumulating in f32.
                attnT = ew.tile([P, TCF, 512], F8, name="attnT")
                for jc in range(TCF):
                    i0 = max(0, (jc - 4) * P)  # queries < i0 fully masked
                    cp = PS()
                    nc.tensor.matmul(cp[:, i0:512],
                                     lhsT=kh[:, jc * P:(jc + 1) * P],
                                     rhs=quh[:, i0:512],
                                     start=True, stop=False,
                                     skip_group_check=True)
                    for ic in range(TCC):
                        nc.tensor.matmul(cp[:, ic * P:(ic + 1) * P],
                                         lhsT=shp[:, ic, jc * P:(jc + 1) * P],
                                         rhs=ident8,
                                         start=(ic * P < i0),
                                         stop=(ic == TCC - 1),
                                         skip_group_check=True)
                    nc.scalar.activation(out=attnT[:, jc, :], in_=cp,
                                         func=Act.Exp, scale=SCALE, bias=ln4n)
                dnp = PS()
                for i in range(TCF // 2):
                    nc.tensor.matmul(dnp[0:64, :], lhsT=oneshalf8,
                                     rhs=attnT[:, 2 * i:2 * i + 2, :],
                                     start=(i == 0), stop=(i == TCF // 2 - 1),
                                     perf_mode=DR)
                recip = rw.tile([1, 512], F32, name="recip")
                nc.vector.reciprocal(out=recip, in_=dnp[0:1, :])
                recipB = rw.tile([HEAD_DIM, 512], F32, name="recipB")
                nc.gpsimd.partition_broadcast(recipB, recip)
                avp = PS()
                for i in range(TCF // 2):
                    nc.tensor.matmul(
                        avp[0:HEAD_DIM, :],
                        lhsT=v8[:, 2 * i:2 * i + 2, h * HEAD_DIM:(h + 1) * HEAD_DIM],
                        rhs=attnT[:, 2 * i:2 * i + 2, :],
                        start=(i == 0), stop=(i == TCF // 2 - 1), perf_mode=DR)
                nc.vector.tensor_tensor(out=av8[rb:rb + HEAD_DIM, ch, :],
                                        in0=avp[0:HEAD_DIM, :], in1=recipB,
                                        op=AluOp.mult)

            shps = {}
            shps[0] = emit_pos(0)
            shps[1] = emit_pos(1)
            shps[2] = emit_pos(2)
            # kT: [feature n, token] layout
            for n in range(DC):
                for th in range(2):
                    ps = PS()
                    _dr_chain(nc, ps, wkv, x1T8, n * P, (n + 1) * P, DC,
                              th * 512, (th + 1) * 512)
                    if n % 2 == 0:
                        nc.scalar.activation(out=kT[:, n, th * 512:(th + 1) * 512],
                                             in_=ps, func=Act.Identity,
                                             bias=csb["bkvK"][:, n:n + 1], scale=DS)
                    else:
                        nc.vector.tensor_scalar(out=kT[:, n, th * 512:(th + 1) * 512],
                                                in0=ps, scalar1=DS,
                                                scalar2=csb["bkvK"][:, n:n + 1],
                                                op0=AluOp.mult, op1=AluOp.add)
            # v8: [token, feature] layout (lhsT = x1T8 token-chunks)
            for t in range(TCF):
                for nh in range(2):
                    ps = PS()
                    for i in range(DC // 2):
                        nc.tensor.matmul(
                            ps, lhsT=x1T8[:, 2 * i:2 * i + 2, t * P:(t + 1) * P],
                            rhs=wkv[:, 2 * i:2 * i + 2,
                                    D + nh * 512:D + (nh + 1) * 512],
                            start=(i == 0), stop=(i == DC // 2 - 1), perf_mode=DR)
                    nc.vector.tensor_scalar_mul(v8[:, t, nh * 512:(nh + 1) * 512],
                                                ps, DS * SV)
            fr_x1T8(); fr_wkv()

            # ---- prefetch (Act HWDGE queue, overlaps attention loop):
            # GRU1 weights + inputs, proj ----
            gru1w = {}
            for m in ("Wr", "Ur", "Wz", "Uz", "Wg", "Ug"):
                w, fr = mk(f"g1w_{m}", (P, DC, D), F8, "left")
                nc.scalar.dma_start(
                    out=w, in_=gw_d[(1, m)][:].rearrange("(kc p) n -> p kc n", p=P))
                gru1w[m] = (w, fr)
            inpTb, fr_inpb = mk("inpTb", (P, DC, CUR), BF16, "left")
            nc.scalar.dma_start(out=inpTb,
                                in_=inpTb_d[:].rearrange("(dc p) t -> p dc t", p=P))
            inpT8, fr_inp8 = mk("inpT8", (P, DC, CUR), F8, "left")
            nc.scalar.dma_start(out=inpT8,
                                in_=inpT8_d[:].rearrange("(dc p) t -> p dc t", p=P))
            wproj, fr_wproj = mk("wproj", (P, DC, D), F8, "left")
            nc.scalar.dma_start(out=wproj,
                                in_=wproj_d[:].rearrange("(kc p) n -> p kc n", p=P))

            for h in range(3, HEAD_NUM + 3):
                if h < HEAD_NUM:
                    shps[h] = emit_pos(h)
                emit_rest(h - 3, shps.pop(h - 3))
        fr_rT(); fr_qvT(); fr_quT(); fr_v8(); fr_kT()

        # ---- right-side long-lived tiles (lifetime order) + prefetch of
        # MLP W2 and GRU2 weights (overlaps proj/GRU1) ----
        o1b, fr_o1b = mk("o1b", (P, DC, CUR), BF16, "right")
        o18, fr_o18 = mk("o18", (P, DC, CUR), F8, "right")
        o2f, fr_o2f = mk("o2f", (P, DC, CUR), F32, "right")
        w2, fr_w2 = mk("w2", (P, HC, D), F8, "right")
        nc.scalar.dma_start(out=w2,
                            in_=w2_d[:].rearrange("(kc p) n -> p kc n", p=P))
        w1, fr_w1 = mk("w1", (P, DC, HID), F8, "right")
        nc.scalar.dma_start(out=w1,
                            in_=w1_d[:].rearrange("(kc p) n -> p kc n", p=P))

        # ================= Phase 4: proj + GRU1 =================
        a1T8, fr_a1T8 = mk("a1T8", (P, DC, CUR), F8, "right")
        for n in range(DC):
            ps = PS()
            _dr_chain(nc, ps, wproj, av8, n * P, (n + 1) * P, DC, 0, CUR)
            nc.scalar.activation(out=a1T8[:, n, :], in_=ps, func=Act.Relu,
                                 bias=csb["bprojs"][:, n:n + 1],
                                 scale=SA / (S_AV * SW))
        fr_wproj()

        with ExitStack() as ph:
            _gru(nc, tc, ph, PS, csb, gru1w, 1, a1T8, inpT8, inpTb, o1b, o18,
                 F32_out=None)
        fr_inp8(); fr_inpb()
        for m in ("Ug", "Wg", "Uz", "Wz", "Ur", "Wr"):
            gru1w[m][1]()
        fr_a1T8()

        # ---- prefetch GRU2 weights (left side; overlaps LN2 + MLP) ----
        gru2w = {}
        for m in ("Wr", "Ur", "Wz", "Uz", "Wg", "Ug"):
            w, fr = mk(f"g2w_{m}", (P, DC, D), F8, "left")
            nc.scalar.dma_start(
                out=w, in_=gw_d[(2, m)][:].rearrange("(kc p) n -> p kc n", p=P))
            gru2w[m] = (w, fr)

        # ================= Phase 5: LN2 =================
        x2T8, fr_x2T8 = mk("x2T8", (P, DC, CUR), F8, "right")
        with ExitStack() as ph:
            lw = ph.enter_context(tc.tile_pool(name="lw2", bufs=2, side="left"))
            sqp = ph.enter_context(tc.tile_pool(name="sqp", bufs=1, side="left"))
            sq = sqp.tile([P, DC, CUR], BF16, name="sq2")
            for n in range(DC):
                nc.gpsimd.tensor_tensor(out=sq[:, n, :], in0=o1b[:, n, :],
                                        in1=o1b[:, n, :], op=AluOp.mult)
            s1p = psum_m.tile([1, 512], F32, name="s1p2", tag="sm")
            for dc in range(DC):
                nc.tensor.matmul(s1p, lhsT=ones_b, rhs=o1b[:, dc, :],
                                 start=(dc == 0), stop=(dc == DC - 1))
            s2p = psum_m.tile([1, 512], F32, name="s2p2", tag="sm")
            for dc in range(DC):
                nc.tensor.matmul(s2p, lhsT=ones_b, rhs=sq[:, dc, :],
                                 start=(dc == 0), stop=(dc == DC - 1))
            mean = lw.tile([1, 512], F32, name="mean2")
            nc.vector.tensor_scalar_mul(mean, s1p, 1.0 / D)
            var = lw.tile([1, 512], F32, name="var2")
            nc.vector.scalar_tensor_tensor(out=var, in0=mean, scalar=1.0,
                                           in1=mean, op0=AluOp.mult, op1=AluOp.mult)
            m2m = lw.tile([1, 512], F32, name="m2m2")
            nc.vector.tensor_scalar_mul(m2m, s2p, 1.0 / D)
            nc.vector.tensor_sub(var, m2m, var)
            sd = lw.tile([1, 512], F32, name="sd2")
            nc.scalar.activation(out=sd, in_=var, func=Act.Sqrt, bias=eps_t)
            rstd = lw.tile([1, 512], F32, name="rstd2")
            nc.vector.reciprocal(out=rstd, in_=sd)
            mean_b = lw.tile([1, 512], BF16, name="mean2b")
            nc.vector.tensor_copy(mean_b, mean)
            rstd_b = lw.tile([1, 512], BF16, name="rstd2b")
            nc.vector.tensor_copy(rstd_b, rstd)
            meanB = lw.tile([P, 512], BF16, name="meanB2")
            nc.gpsimd.partition_broadcast(meanB, mean_b)
            rstdB = lw.tile([P, 512], BF16, name="rstdB2")
            nc.gpsimd.partition_broadcast(rstdB, rstd_b)
            tw = ph.enter_context(tc.tile_pool(name="tw2", bufs=3, side="left"))
            for n in range(DC):
                t1 = tw.tile([P, 512], BF16, name="t1b")
                nc.vector.tensor_sub(t1, o1b[:, n, :], meanB)
                t2 = tw.tile([P, 512], BF16, name="t2b")
                nc.vector.tensor_tensor(out=t2, in0=t1, in1=rstdB, op=AluOp.mult)
                nc.scalar.activation(out=x2T8[:, n, :], in_=t2, func=Act.Identity,
                                     bias=csb["ln2bs"][:, n:n + 1],
                                     scale=csb["ln2gs"][:, n:n + 1])

        # ================= Phase 6: MLP =================
        with ExitStack() as ph6:
            m1w = ph6.enter_context(tc.tile_pool(name="m1w", bufs=1, side="right"))
            m1T8 = m1w.tile([P, HC, 512], F8)
            for n in range(HC):
                ps = PS()
                _dr_chain(nc, ps, w1, x2T8, n * P, (n + 1) * P, DC, 0, CUR)
                nc.scalar.activation(out=m1T8[:, n, :], in_=ps, func=Act.Relu,
                                     bias=csb["b1s"][:, n:n + 1],
                                     scale=DS * SA)
            m2T8, fr_m2T8 = mk("m2T8", (P, DC, CUR), F8, "left")
            for n in range(DC):
                ps = PS()
                _dr_chain(nc, ps, w2, m1T8, n * P, (n + 1) * P, HC, 0, CUR)
                nc.scalar.activation(out=m2T8[:, n, :], in_=ps, func=Act.Relu,
                                     bias=csb["b2s"][:, n:n + 1], scale=DS * SA)
        fr_x2T8(); fr_w1(); fr_w2()

        # ================= Phase 7: GRU2 + out =================
        outr = outT_d[:].rearrange("(dc p) t -> p dc t", p=P)

        def stream_out(n):
            if n % 2 == 1:
                nc.sync.dma_start(out=outr[:, n - 1:n + 1, :],
                                  in_=o2f[:, n - 1:n + 1, :])

        with ExitStack() as ph:
            _gru(nc, tc, ph, PS, csb, gru2w, 2, m2T8, o18, o1b, None, None,
                 F32_out=o2f, stream_out=stream_out)
        fr_m2T8()
        for m in ("Ug", "Wg", "Uz", "Wz", "Ur", "Wr"):
            gru2w[m][1]()
        fr_o2f(); fr_o18(); fr_o1b(); fr_av8()


def _gru(nc, tc, ph, PS, csb, gw, g, yT8, xT8, xTb, ob, o8, F32_out,
         stream_out=None):
    """GRU gate: yT8/xT8 fp8 gemm inputs (scale SA), xTb bf16 elementwise path.
    Writes ob (bf16) + o8 (fp8) or F32_out. gw: preloaded weight tiles
    {name: (tile, free_fn)}."""
    gtmp = ph.enter_context(tc.tile_pool(name=f"gt{g}", bufs=3, side="left"))
    gper = ph.enter_context(tc.tile_pool(name=f"gp{g}", bufs=1, side="left"))

    def gate_psum(ps, w, u, n, rhs2):
        for i in range(DC // 2):
            nc.tensor.matmul(ps, lhsT=w[:, 2 * i:2 * i + 2, n * P:(n + 1) * P],
                             rhs=yT8[:, 2 * i:2 * i + 2, :],
                             start=(i == 0), stop=False, perf_mode=DR)
        for i in range(DC // 2):
            nc.tensor.matmul(ps, lhsT=u[:, 2 * i:2 * i + 2, n * P:(n + 1) * P],
                             rhs=rhs2[:, 2 * i:2 * i + 2, :],
                             start=False, stop=(i == DC // 2 - 1), perf_mode=DR)

    wr, ur = gw["Wr"][0], gw["Ur"][0]
    wz, uz = gw["Wz"][0], gw["Uz"][0]
    wg, ug = gw["Wg"][0], gw["Ug"][0]
    rx8 = gper.tile([P, DC, 512], F8, name="rx8")
    for n in range(DC):
        ps = PS()
        gate_psum(ps, wr, ur, n, xT8)
        rr = gtmp.tile([P, 512], BF16, name="rr")
        nc.scalar.activation(out=rr, in_=ps, func=Act.Sigmoid, scale=DS,
                             bias=csb["zerob"])
        nc.gpsimd.tensor_tensor(out=rx8[:, n, :], in0=rr, in1=xT8[:, n, :],
                                op=AluOp.mult)
    zt = gper.tile([P, DC, 512], BF16, name="zt")
    for n in range(DC):
        ps = PS()
        gate_psum(ps, wz, uz, n, xT8)
        nc.scalar.activation(out=zt[:, n, :], in_=ps, func=Act.Sigmoid,
                             bias=csb[f"nbg{g}"][:, n:n + 1], scale=DS)
    for n in range(DC):
        ps = PS()
        gate_psum(ps, wg, ug, n, rx8)
        ht = gtmp.tile([P, 512], BF16, name="ht")
        nc.scalar.activation(out=ht, in_=ps, func=Act.Tanh, scale=DS,
                             bias=csb["zerob"])
        t1 = gtmp.tile([P, 512], BF16, name="tg1")
        nc.gpsimd.tensor_tensor(out=t1, in0=ht, in1=xTb[:, n, :],
                                op=AluOp.subtract)
        t2 = gtmp.tile([P, 512], BF16, name="tg2")
        nc.gpsimd.tensor_tensor(out=t2, in0=t1, in1=zt[:, n, :], op=AluOp.mult)
        if F32_out is not None:
            nc.vector.tensor_tensor(out=F32_out[:, n, :], in0=t2,
                                    in1=xTb[:, n, :], op=AluOp.add)
            if stream_out is not None:
                stream_out(n)
        else:
            nc.vector.tensor_tensor(out=ob[:, n, :], in0=t2, in1=xTb[:, n, :],
                                    op=AluOp.add)
            nc.gpsimd.tensor_scalar(out=o8[:, n, :], in0=ob[:, n, :],
                                    scalar1=SA, scalar2=0.0,
                                    op0=AluOp.mult, op1=AluOp.add)


_NC_CACHE = {}


def _get_nc():
    if "nc" not in _NC_CACHE:
        _NC_CACHE["nc"] = _build()
    return _NC_CACHE["nc"]


def _chunk_t(vec):
    n = vec.shape[0] // P
    return np.ascontiguousarray(vec.reshape(n, P).T.astype(np.float32))


def _f8(x, s):
    y = np.asarray(x, np.float32) * s
    return np.clip(y, -224.0, 224.0).astype(NF8)


def _prep(inputs):
    f32 = np.float32
    inp = np.asarray(inputs["inputs"], f32)
    mem = np.asarray(inputs["memory"], f32)
    pos = np.asarray(inputs["pos_embedding"], f32)[:, 0, :]
    bkv = np.asarray(inputs["bkv"], f32)
    u = np.asarray(inputs["u"], f32).reshape(-1)
    v = np.asarray(inputs["v"], f32).reshape(-1)
    bq = np.asarray(inputs["bq"], f32)
    bproj_eff = (np.asarray(inputs["bproj"], f32)
                 + bkv[D:] @ np.asarray(inputs["Wproj"], f32))

    cm = np.concatenate([
        _chunk_t(np.asarray(inputs["ln1_g"], f32) * SA),
        _chunk_t(np.asarray(inputs["ln1_b"], f32) * SA),
        _chunk_t(np.asarray(inputs["ln2_g"], f32) * SA),
        _chunk_t(np.asarray(inputs["ln2_b"], f32) * SA),
        _chunk_t(bkv[:D]),
        _chunk_t(bq + u),
        _chunk_t(bq + v),
        _chunk_t(np.asarray(inputs["bpos"], f32)),
        _chunk_t(bproj_eff * SA),
        _chunk_t(np.asarray(inputs["mlp_b1"], f32) * SA),
        _chunk_t(np.asarray(inputs["mlp_b2"], f32) * SA),
        _chunk_t(-np.asarray(inputs["g1_bg"], f32)),
        _chunk_t(-np.asarray(inputs["g2_bg"], f32)),
        _chunk_t(v - u),
    ], axis=1)
    shared = {
        "posT8": _f8(np.ascontiguousarray(pos.T), SA),
        "cmerged": np.ascontiguousarray(cm),
        "Wkv8": _f8(inputs["Wkv"], SW),
        "Wq8": _f8(inputs["Wq"], SW),
        "Wpos8": _f8(inputs["Wpos"], SW),
        "Wproj8": _f8(inputs["Wproj"], SW),
        "mlp_W18": _f8(inputs["mlp_W1"], SW),
        "mlp_W28": _f8(inputs["mlp_W2"], SW),
    }
    for g in (1, 2):
        for m in ("Wr", "Ur", "Wz", "Uz", "Wg", "Ug"):
            shared[f"g{g}_{m}8"] = _f8(inputs[f"g{g}_{m}"], SW)

    in_maps = []
    for b in range(BS):
        im = dict(shared)
        xb = np.concatenate([mem[:, b, :], inp[:, b, :]], axis=0)
        im["xT_b"] = np.ascontiguousarray(xb.T).astype(NBF)
        im["inpT_b"] = np.ascontiguousarray(inp[:, b, :].T).astype(NBF)
        im["inpT8"] = _f8(np.ascontiguousarray(inp[:, b, :].T), SA)
        in_maps.append(im)
    return in_maps


def kernel(**inputs):
    nc = _get_nc()
    in_maps = _prep(inputs)
    res = run_bass_kernel_spmd(nc, in_maps, core_ids=list(range(BS)))
    out = np.stack([res.results[b]["outT"].T for b in range(BS)], axis=1)
    return np.ascontiguousarray(out.astype(np.float32))


if __name__ == "__main__":
    _get_nc()
    print("build+compile OK")

